# revision 2
# baseline (speedup 1.0000x reference)
"""Llama GQA attention (B=1, S=2048, H=2048, 32 q heads / 8 kv heads, hd=64)
on 8 Trainium2 NeuronCores.

Sharding: core i owns kv head i and query heads 4i..4i+3 (tensor parallel over
heads). Each core computes QKV projections (full seq, its heads), RoPE, scores
transposed S_T[k, q] = K @ Q^T per head, softmax via exp (no max subtraction —
|scores| is small) with the denominator obtained for free by appending a ones
column to V, then out_T = V_ext^T @ E_T. The per-core attention outputs
[256 feat, 2048 seq] are exchanged with an AllToAll so that core j ends up
with the full 2048-feature activation for seq slice [256j, 256j+256), on which
it runs the o_proj (out_T = Wo^T.T @ attn_T). Host reassembles/transposes.

attn_weights are written per-core as attn_t[h, k, q] (transposed layout, which
is both the natural matmul layout and a contiguous DMA); the host transposes
back. Upper-triangle (masked) entries are exactly 0 and are never computed —
the device output buffer is zero-initialised by the runtime.

Matmuls run in float32r (TRN2 full-rate fp32 mode, ~1e-3 rel err), psum
accumulation in fp32.
"""

import numpy as np

S = 2048
H = 2048
NH = 32
KVH = 8
HD = 64
NCORES = 8
HPC = NH // KVH  # q heads per core = 4
QC = 1024  # q-chunk width in phase B
SCALE = HD ** -0.5
NEG = -1e9

_CACHE = {}


def _build(causal: bool):
    import concourse.bacc as bacc
    import concourse.mybir as mybir
    import concourse.tile as tile
    from concourse.masks import make_identity

    F32 = mybir.dt.float32
    F32R = mybir.dt.float32r
    EXP = mybir.ActivationFunctionType.Exp

    nc = bacc.Bacc("TRN2", target_bir_lowering=False, debug=False,
                   num_devices=NCORES)

    hsT = nc.dram_tensor("hsT", [H, S], F32R, kind="ExternalInput")
    wqkvT = nc.dram_tensor("wqkvT", [H, 384], F32R, kind="ExternalInput")
    woT = nc.dram_tensor("woT", [H, H], F32R, kind="ExternalInput")
    cosS = nc.dram_tensor("cosS", [128, S], F32, kind="ExternalInput")
    sinP = nc.dram_tensor("sinP", [128, S], F32, kind="ExternalInput")
    if not causal:
        maskT = nc.dram_tensor("maskT", [S, S], F32, kind="ExternalInput")

    attn_t = nc.dram_tensor("attn_t", [HPC, S, S], F32, kind="ExternalOutput")
    out_t = nc.dram_tensor("out_t", [H, S // NCORES], F32, kind="ExternalOutput")

    NSB = S // 128  # 16 seq blocks of 128
    NHB = H // 128  # 16 hidden blocks

    with tile.TileContext(nc) as tc:
        with tc.tile_pool(name="pers", bufs=1) as pers:
            # persistent SBUF state
            qt_rope = pers.tile([128, 2, S], F32R)   # q heads, pairs (2t+hi)
            kt_stack = pers.tile([128, S], F32R)     # kv head, duplicated rows
            v_ext = pers.tile([128, NSB, 68], F32R)  # V^T per seq block + ones col
            ot_all = pers.tile([128, 2, S], F32R)     # normalized attn out_T
            tri = pers.tile([128, 128], F32)         # additive diag-block mask
            ident64 = pers.tile([64, 64], F32)
            ones_row = pers.tile([1, 128], F32R)     # for denom broadcast

            # constants
            make_identity(nc, ident64)
            nc.gpsimd.memset(tri, 0.0)
            # tri[k_r, q_c] = 0 if q_c >= k_r else NEG  (iota = c - r)
            nc.gpsimd.affine_select(
                out=tri, in_=tri, compare_op=mybir.AluOpType.is_ge,
                fill=NEG, base=0, pattern=[[1, 128]], channel_multiplier=-1,
            )
            ones32 = pers.tile([1, 128], F32)
            nc.vector.memset(ones32, 1.0)
            nc.vector.tensor_copy(out=ones_row, in_=ones32)
            onesP = pers.tile([128, NSB], F32)
            nc.vector.memset(onesP, 1.0)
            nc.vector.tensor_copy(out=v_ext[:, :, 64:65], in_=onesP)

            # ---------------- Phase A: QKV projection + RoPE + V transpose
            with (
                tc.tile_pool(name="pA", bufs=1) as pA,
                tc.tile_pool(name="psA", bufs=1, space="PSUM") as psA,
            ):
                wqkv_s = pA.tile([128, NHB, 384], F32R)
                nc.sync.dma_start(
                    out=wqkv_s, in_=wqkvT.rearrange("(b p) n -> p b n", p=128))
                cos_s = pA.tile([128, S], F32)
                nc.sync.dma_start(out=cos_s, in_=cosS[:, :])
                sinp_s = pA.tile([128, S], F32)
                nc.sync.dma_start(out=sinp_s, in_=sinP[:, :])

                for sc in range(4):  # seq chunks of 512
                    cols = slice(512 * sc, 512 * sc + 512)
                    psq = psA.tile([128, 1024], F32, tag="psq", bufs=2)
                    pskv = psA.tile([128, 512], F32, tag="pskv", bufs=2)
                    for hb in range(NHB):
                        hst = pA.tile([128, 512], F32R, tag="hst", bufs=4)
                        nc.sync.dma_start(
                            out=hst,
                            in_=hsT[128 * hb:128 * hb + 128, cols])
                        st = (hb == 0)
                        sp = (hb == NHB - 1)
                        nc.tensor.matmul(psq[:, 0:512], wqkv_s[:, hb, 0:128],
                                         hst, start=st, stop=sp)
                        nc.tensor.matmul(psq[:, 512:1024],
                                         wqkv_s[:, hb, 128:256],
                                         hst, start=st, stop=sp)
                        nc.tensor.matmul(pskv, wqkv_s[:, hb, 256:384],
                                         hst, start=st, stop=sp)

                    # RoPE on q head pairs
                    for t in range(2):
                        ph = psq[:, 512 * t:512 * t + 512]
                        t1 = pA.tile([128, 512], F32, tag="t1", bufs=2)
                        nc.vector.tensor_mul(out=t1, in0=ph, in1=cos_s[:, cols])
                        rot = pA.tile([128, 512], F32, tag="rot", bufs=2)
                        for g in range(4):
                            ob = 32 * g
                            sb_ = ob ^ 32
                            nc.vector.tensor_mul(
                                out=rot[ob:ob + 32, :],
                                in0=ph[sb_:sb_ + 32, :],
                                in1=sinp_s[ob:ob + 32, cols])
                        nc.vector.tensor_add(
                            out=qt_rope[:, t, cols], in0=t1, in1=rot)
                    # RoPE on k (rows 0:64 of pskv)
                    t1k = pA.tile([64, 512], F32, tag="t1k", bufs=2)
                    nc.vector.tensor_mul(out=t1k, in0=pskv[0:64, :],
                                         in1=cos_s[0:64, cols])
                    rotk = pA.tile([64, 512], F32, tag="rotk", bufs=2)
                    for g in range(2):
                        ob = 32 * g
                        sb_ = ob ^ 32
                        nc.vector.tensor_mul(
                            out=rotk[ob:ob + 32, :],
                            in0=pskv[sb_:sb_ + 32, :],
                            in1=sinp_s[ob:ob + 32, cols])
                    nc.vector.tensor_add(
                        out=kt_stack[0:64, cols], in0=t1k, in1=rotk)
                    nc.gpsimd.tensor_copy(
                        out=kt_stack[64:128, cols], in_=kt_stack[0:64, cols])
                    # V transpose (rows 64:128 of pskv)
                    v_sb = pA.tile([64, 512], F32, tag="vsb", bufs=2)
                    nc.vector.tensor_copy(out=v_sb, in_=pskv[64:128, :])
                    for j in range(4):
                        vt_ps = psA.tile([128, 64], F32, tag="vtr", bufs=2)
                        nc.tensor.transpose(
                            vt_ps, v_sb[:, 128 * j:128 * j + 128], ident64)
                        nc.vector.tensor_copy(
                            out=v_ext[:, 4 * sc + j, 0:64], in_=vt_ps)

            # ---------------- Phase B: attention per head / q-chunk
            with (
                tc.tile_pool(name="pB", bufs=1) as pB,
                tc.tile_pool(name="psB", bufs=1, space="PSUM") as psB,
            ):
                for h in range(HPC):
                    hb64 = 64 * (h % 2)
                    pair = h // 2
                    for c in range(S // QC):
                        kmax = (QC // 128) * (c + 1) if causal else NSB
                        q0 = QC * c
                        ov = psB.tile([65, QC], F32, tag="ov", bufs=2)
                        et = {}
                        for kb in range(kmax):
                            lo = max(0, 128 * kb - q0) if causal else 0
                            st_ps = psB.tile([128, QC], F32, tag="st", bufs=2)
                            for half in range(2):
                                a = max(lo, 512 * half)
                                b = 512 * half + 512
                                if a >= b:
                                    continue
                                nc.tensor.matmul(
                                    st_ps[:, a:b],
                                    kt_stack[hb64:hb64 + 64,
                                             128 * kb:128 * kb + 128],
                                    qt_rope[hb64:hb64 + 64, pair,
                                            q0 + a:q0 + b],
                                    start=True, stop=True)
                            if causal:
                                if 128 * kb >= q0:
                                    nc.vector.tensor_add(
                                        out=st_ps[:, lo:lo + 128],
                                        in0=st_ps[:, lo:lo + 128], in1=tri)
                            else:
                                mt = pB.tile([128, QC], F32, tag="mt", bufs=2)
                                nc.sync.dma_start(
                                    out=mt,
                                    in_=maskT[128 * kb:128 * kb + 128,
                                              q0:q0 + QC])
                                nc.vector.tensor_add(
                                    out=st_ps, in0=st_ps, in1=mt)
                            e = pB.tile([128, QC], F32R, tag=f"et{kb}", bufs=1,
                                        name=f"et{kb}")
                            et[kb] = e
                            nc.scalar.activation(
                                out=e[:, lo:QC], in_=st_ps[:, lo:QC],
                                func=EXP, scale=SCALE)
                            for half in range(2):
                                a = max(lo, 512 * half)
                                b = 512 * half + 512
                                if a >= b:
                                    continue
                                nc.tensor.matmul(
                                    ov[:, a:b], v_ext[:, kb, 0:65],
                                    e[:, a:b],
                                    start=(kb == 0), stop=(kb == kmax - 1),
                                    skip_group_check=True)

                        # softmax denominator -> broadcast tile
                        dinv = pB.tile([1, QC], F32R, tag="dinv", bufs=2)
                        with nc.allow_low_precision(
                                reason="f32r rounding of softmax denom"):
                            nc.vector.reciprocal(out=dinv, in_=ov[64:65, :])
                        db_ps = psB.tile([128, QC], F32, tag="st", bufs=2)
                        for half in range(2):
                            nc.tensor.matmul(
                                db_ps[:, 512 * half:512 * half + 512],
                                ones_row,
                                dinv[:, 512 * half:512 * half + 512],
                                start=True, stop=True)
                        db_sb = pB.tile([128, QC], F32, tag="db", bufs=2)
                        nc.vector.tensor_copy(out=db_sb, in_=db_ps)

                        # normalize + write attn weights (transposed layout)
                        for kb in range(kmax):
                            lo = max(0, 128 * kb - q0) if causal else 0
                            aw = pB.tile([128, QC], F32, tag="aw", bufs=4)
                            nc.vector.tensor_mul(
                                out=aw[:, lo:QC], in0=et[kb][:, lo:QC],
                                in1=db_sb[:, lo:QC])
                            nc.sync.dma_start(
                                out=attn_t[h, 128 * kb:128 * kb + 128,
                                           q0 + lo:q0 + QC],
                                in_=aw[:, lo:QC])

                        # normalize attention output
                        oc = pB.tile([64, QC], F32R, tag="oc", bufs=2)
                        nc.vector.tensor_mul(
                            out=oc, in0=ov[0:64, :], in1=db_sb[0:64, :])
                        nc.gpsimd.tensor_copy(
                            out=ot_all[hb64:hb64 + 64, pair, q0:q0 + QC],
                            in_=oc)

            # ---------------- Phase C: AllToAll + o_proj
            with (
                tc.tile_pool(name="pC", bufs=1) as pC,
                tc.tile_pool(name="psC", bufs=1, space="PSUM") as psC,
                tc.tile_pool(name="dramC", bufs=1, space="DRAM") as dramC,
            ):
                SJ = S // NCORES  # 256
                a2a_in = dramC.tile([H, SJ], F32R)
                a2a_out = dramC.tile([H, SJ], F32R)
                for j in range(NCORES):
                    for t in range(2):
                        nc.sync.dma_start(
                            out=a2a_in[256 * j + 128 * t:
                                       256 * j + 128 * t + 128, :],
                            in_=ot_all[:, t, SJ * j:SJ * j + SJ])
                import concourse.mybir as _mybir
                nc.gpsimd.collective_compute(
                    "AllToAll", _mybir.AluOpType.bypass,
                    replica_groups=[list(range(NCORES))],
                    ins=[a2a_in.opt()], outs=[a2a_out.opt()],
                )
                ag_sb = pC.tile([128, NHB, SJ], F32R)
                nc.sync.dma_start(
                    out=ag_sb,
                    in_=a2a_out.rearrange("(b p) s -> p b s", p=128))
                woT_r = woT.rearrange("(b p) n -> p b n", p=128)
                for nb in range(NHB):
                    wo_sb = pC.tile([128, NHB, 128], F32R, tag="wo", bufs=3)
                    nc.sync.dma_start(
                        out=wo_sb,
                        in_=woT_r[:, :, 128 * nb:128 * nb + 128])
                    ps_o = psC.tile([128, SJ], F32, tag="po", bufs=2)
                    for fb in range(NHB):
                        nc.tensor.matmul(
                            ps_o, wo_sb[:, fb, :], ag_sb[:, fb, :],
                            start=(fb == 0), stop=(fb == NHB - 1))
                    o_sb = pC.tile([128, SJ], F32, tag="os", bufs=3)
                    nc.vector.tensor_copy(out=o_sb, in_=ps_o)
                    nc.sync.dma_start(
                        out=out_t[128 * nb:128 * nb + 128, :], in_=o_sb)

    nc.compile()
    return nc


def _prep_host(hidden_states, cos, sin, attention_mask, Wq, Wk, Wv, Wo):
    f32 = np.float32
    hs = np.ascontiguousarray(hidden_states.reshape(S, H), dtype=f32)
    hsT = np.ascontiguousarray(hs.T)
    cos2 = np.asarray(cos, dtype=f32).reshape(S, HD)
    sin2 = np.asarray(sin, dtype=f32).reshape(S, HD)
    cosT = np.ascontiguousarray(cos2.T)  # [64, S]
    sinT = np.ascontiguousarray(sin2.T)
    cosS = np.concatenate([cosT, cosT], axis=0)  # [128, S]
    sinP64 = np.concatenate([-sinT[0:32], sinT[32:64]], axis=0)
    sinP = np.concatenate([sinP64, sinP64], axis=0)  # [128, S]
    woT = np.ascontiguousarray(np.asarray(Wo, dtype=f32).T)

    mask = np.asarray(attention_mask, dtype=f32).reshape(S, S)
    tril = np.tril(np.ones((S, S), dtype=bool))
    causal = bool(
        np.all(mask[tril] == 0.0) and np.all(mask[~tril] <= -1e8))
    maskT = None
    if not causal:
        maskT = np.ascontiguousarray((mask * (1.0 / SCALE)).T)

    in_maps = []
    for i in range(NCORES):
        wq_i = Wq[256 * i:256 * i + 256]
        wk_i = Wk[64 * i:64 * i + 64]
        wv_i = Wv[64 * i:64 * i + 64]
        wqkvT = np.ascontiguousarray(
            np.concatenate([wq_i, wk_i, wv_i], axis=0).T.astype(f32))
        m = {"hsT": hsT, "wqkvT": wqkvT, "woT": woT,
             "cosS": cosS, "sinP": sinP}
        if not causal:
            m["maskT"] = maskT
        in_maps.append(m)
    return causal, in_maps


def kernel(hidden_states, cos, sin, attention_mask, Wq, Wk, Wv, Wo):
    from concourse import bass_utils

    causal, in_maps = _prep_host(
        hidden_states, cos, sin, attention_mask, Wq, Wk, Wv, Wo)

    if causal not in _CACHE:
        _CACHE[causal] = _build(causal)
    nc = _CACHE[causal]

    res = bass_utils.run_bass_kernel_spmd(
        nc, in_maps, core_ids=list(range(NCORES)))

    out = np.empty((1, S, H), dtype=np.float32)
    attn = np.empty((1, NH, S, S), dtype=np.float32)
    SJ = S // NCORES
    for i in range(NCORES):
        r = res.results[i]
        out[0, SJ * i:SJ * i + SJ, :] = r["out_t"].T
        at = r["attn_t"]
        for h in range(HPC):
            attn[0, HPC * i + h] = at[h].T
    return out, attn
